# revision 13
# baseline (speedup 1.0000x reference)
"""Trainium2 Bass kernel for nn_CausalLSTMNodeCell (B=1048576, D=32, H=16, C=3).

Strategy: pure data parallel over the batch across 8 cores, with output rows
organized by residue b mod 3 so the TF-row-major child_r reshape becomes three
extra contiguous-row matmuls. Per 128-row block (K=98, block-diagonal rhs):
  psum[:, 0:32]   = xtB.T @ WB[rho]   (r1, r2 gates)
  psum[:, 32:144] = xtA.T @ WA[rho]   (ifo, n1, n2, r0, a)
All gates (incl. tanh'd "a", via tanh z = 2*sigmoid(2z)-1 with host-prescaled
weights) come out of one Sigmoid op per psum group on ScalarE; elementwise
recombination runs in f32 on VectorE with three muls offloaded to GPSIMD.
Matmul inputs bf16 (host pre-cast + pre-transposed); biases folded in via
ones-rows of the stationary operand. c_prev/child_n ship as bf16; n,h,c are
stored bf16 and upcast on host. Supergroups of 32 blocks are software-
pipelined with LAG=1 (gates phase of s overlaps elementwise of s-1); loads
ride the SP HWDGE ring, stores the ACT ring, to avoid head-of-line blocking.
"""

import numpy as np
import ml_dtypes

B, D, H, C = 1048576, 32, 16, 3
NCORES = 8
R = B // NCORES            # 131072 rows per core
TPB = 342                  # blocks per residue section (padded, 43776>=43691)
TP = TPB * 128             # rows per section
NBLK = 3 * TPB             # blocks per core
SG = 32                    # max blocks per supergroup
GRP = 8                    # blocks per psum group
# supergroup table: per residue section, 10 supergroups of 32 + 1 of 22
SG_TABLE = []              # (blk0, size) pairs
for _rho in range(3):
    _off = _rho * TPB
    for _i in range(TPB // SG):
        SG_TABLE.append((_off + _i * SG, SG))
    if TPB % SG:
        SG_TABLE.append((_off + (TPB // SG) * SG, TPB % SG))
NSG = len(SG_TABLE)
KDIM = 98
bf16 = ml_dtypes.bfloat16

CHILD16 = True             # ship child_n as bf16
CP16 = True                # ship c_prev as bf16
OUT16 = True               # store n,h,c as bf16
GPS_OPS = 3                # how many elementwise ops to push to GPSIMD (0..5)
CGRP = False               # c-chain + tanh_c at psum-group granularity
SPLIT_LOADS = False        # split xt loads in halves for finer deps
XT_BUFS = 3                # buffer depth for xtA/xtB load tiles
AXD_BUFS = 4               # buffer depth for c_prev/child load tiles
CCHAIN_POOL = False        # c-chain (ops 1-3) on GPSIMD instead of DVE
GATES16 = False            # store gate activations as bf16
LAG = 1                    # software-pipeline depth (supergroups)

_NC_CACHE = {}


def _build_w(inputs):
    W = np.zeros((49, 144), np.float32)

    def put(cols, wx, wh, bx, bh):
        W[0:32, cols] = inputs[wx]
        W[32:48, cols] = inputs[wh]
        W[48, cols] = inputs[bx] + inputs[bh]

    put(slice(0, 48), "W_ifo_x", "W_ifo_h", "b_ifo_x", "b_ifo_h")
    put(slice(48, 64), "W_n1_x", "W_n1_h", "b_n1_x", "b_n1_h")
    put(slice(64, 80), "W_n2_x", "W_n2_h", "b_n2_x", "b_n2_h")
    put(slice(80, 96), "W_a_x", "W_a_h", "b_a_x", "b_a_h")
    put(slice(96, 144), "W_r_x", "W_r_h", "b_r_x", "b_r_h")
    return W


def host_prep(inputs):
    x = np.asarray(inputs["inputs"], np.float32)
    hp = np.asarray(inputs["h_prev"], np.float32)
    cp = np.asarray(inputs["c_prev"], np.float32)
    ch = np.asarray(inputs["child_n"], np.float32)
    W = _build_w(inputs)
    Wr = W[:, 96:144]
    xh = np.zeros((B + 1, 49), np.float32)
    xh[:B, 0:32] = x
    xh[:B, 32:48] = hp
    xh[:B, 48] = 1.0
    xh16 = xh.astype(bf16)

    chdt = bf16 if CHILD16 else np.float32
    chall = np.empty((B, 48), chdt)
    chall[:, 0:16] = ch[0].astype(chdt)
    chall[:, 16:32] = ch[1].astype(chdt)
    chall[:, 32:48] = ch[2].astype(chdt)

    cores = []
    for m in range(NCORES):
        xtA = np.zeros((KDIM, 3 * TP), bf16)
        xtB = np.zeros((KDIM, 3 * TP), bf16)
        cpp = np.zeros((3 * TP, 16), bf16)
        chp = np.zeros((3 * TP, 48), chdt)
        WA = np.zeros((3, KDIM, 112), np.float32)
        WB = np.zeros((3, KDIM, 32), np.float32)
        for rho in range(3):
            first = m * R + ((rho - m * R) % 3)
            T = len(range(first, (m + 1) * R, 3))
            sl = slice(rho * TP, rho * TP + TP)
            bidx = first + 3 * np.arange(TP)
            bidx = np.minimum(bidx, B)
            bidx[T:] = B
            xtA[0:49, sl] = xh16[bidx].T
            cpp[sl.start:sl.start + T] = cp[first:(m + 1) * R:3]
            chp[sl.start:sl.start + T] = chall[first:(m + 1) * R:3]
            q = [(k * 16 * B + 16 * first) // 48 for k in range(3)]
            c = [16 * ((k + rho) % 3) for k in range(3)]
            for k, dst, rows in ((0, xtA, slice(49, 98)),
                                 (1, xtB, slice(0, 49)),
                                 (2, xtB, slice(49, 98))):
                qi = np.minimum(q[k] + np.arange(TP), B)
                dst[rows, sl] = xh16[qi].T
            WA[rho, 0:49, 0:48] = W[:, 0:48]            # ifo  -> psum 32:80
            WA[rho, 0:49, 48:64] = W[:, 48:64]          # n1   -> 80:96
            WA[rho, 0:49, 64:80] = W[:, 64:80]          # n2   -> 96:112
            WA[rho, 49:98, 80:96] = Wr[:, c[0]:c[0] + 16]   # r0 -> 112:128
            # a-preact scaled by 2: tanh(z) = 2*sigmoid(2z) - 1, so the a
            # column rides the sigmoid activation with a cheap fixup
            WA[rho, 0:49, 96:112] = 2.0 * W[:, 80:96]   # a    -> 128:144
            WB[rho, 0:49, 0:16] = Wr[:, c[1]:c[1] + 16]     # r1 -> 0:16
            WB[rho, 49:98, 16:32] = Wr[:, c[2]:c[2] + 16]   # r2 -> 16:32
        aux = np.concatenate([cpp, chp], axis=1)        # [3TP, 64] bf16
        aux = np.ascontiguousarray(
            aux.reshape(NBLK, 128, 64).transpose(1, 0, 2))
        cores.append(dict(xta=xtA, xtb=xtB, aux=aux,
                          wa=WA.astype(bf16), wb=WB.astype(bf16)))
    return cores


def build_nc(niter=1, sg_bufs=3):
    import concourse.tile as tile
    from concourse import bacc, mybir

    f32 = mybir.dt.float32
    b16 = mybir.dt.bfloat16
    chdt = b16 if CHILD16 else f32
    cpdt = b16 if CP16 else f32
    odt = b16 if OUT16 else f32
    AF = mybir.ActivationFunctionType

    nc = bacc.Bacc(None, target_bir_lowering=False)
    xta_d = nc.dram_tensor("xta", [KDIM, 3 * TP], b16, kind="ExternalInput")
    xtb_d = nc.dram_tensor("xtb", [KDIM, 3 * TP], b16, kind="ExternalInput")
    wa_d = nc.dram_tensor("wa", [3, KDIM, 112], b16, kind="ExternalInput")
    wb_d = nc.dram_tensor("wb", [3, KDIM, 32], b16, kind="ExternalInput")
    aux_d = nc.dram_tensor("aux", [128, NBLK, 64], b16, kind="ExternalInput")
    res_d = nc.dram_tensor("res", [128, NBLK, 48], odt, kind="ExternalOutput")

    # gate columns in psum/GATES:
    R1, R2 = slice(0, 16), slice(16, 32)
    I, F, O = slice(32, 48), slice(48, 64), slice(64, 80)
    N1, N2, R0 = slice(80, 96), slice(96, 112), slice(112, 128)
    A = slice(128, 144)
    CH0, CH1, CH2 = (slice(16 * i + 16, 16 * i + 32) for i in range(3))
    S0, S1, S2, S3, S4, S5 = (slice(16 * i, 16 * i + 16) for i in range(6))
    RN, RH, RC = (slice(16 * i, 16 * i + 16) for i in range(3))
    ALU = mybir.AluOpType

    with tile.TileContext(nc) as tc:
        with (
            tc.tile_pool(name="wp", bufs=1) as wp,
            tc.tile_pool(name="xtab", bufs=XT_BUFS) as xtabp,
            tc.tile_pool(name="axd", bufs=AXD_BUFS) as axdp,
            tc.tile_pool(name="gates", bufs=sg_bufs) as gatesp,
            tc.tile_pool(name="tmp", bufs=sg_bufs) as tmpp,
            tc.tile_pool(name="res", bufs=sg_bufs) as resp,
            tc.tile_pool(name="ps", bufs=2, space="PSUM") as psp,
        ):
            wa_t = wp.tile([KDIM, 3, 112], b16, tag="wa")
            wb_t = wp.tile([KDIM, 3, 32], b16, tag="wb")
            for rho in range(3):
                nc.sync.dma_start(wa_t[:, rho, :], wa_d[rho])
                nc.sync.dma_start(wb_t[:, rho, :], wb_d[rho])

            V = nc.vector
            G = nc.gpsimd
            E6 = G if GPS_OPS >= 1 else V
            E7 = G if GPS_OPS >= 2 else V
            E8 = G if GPS_OPS >= 3 else V
            E9 = G if GPS_OPS >= 4 else V
            E10 = G if GPS_OPS >= 5 else V
            EC = G if CCHAIN_POOL else V

            def gate_phase(s):
                blk0, sz = SG_TABLE[s]
                rho = blk0 // TPB
                col0 = blk0 * 128
                xta_t = xtabp.tile([KDIM, sz * 128], b16, tag="xta")
                nc.sync.dma_start(xta_t[:], xta_d[:, col0:col0 + sz * 128])
                xtb_t = xtabp.tile([KDIM, sz * 128], b16, tag="xtb")
                nc.sync.dma_start(xtb_t[:], xtb_d[:, col0:col0 + sz * 128])
                auxt = axdp.tile([128, sz, 64], b16, tag="aux")
                nc.sync.dma_start(auxt[:], aux_d[:, blk0:blk0 + sz, :])

                gates = gatesp.tile([128, sz, 144],
                                    b16 if GATES16 else f32, tag="gates")
                tmp = tmpp.tile([128, sz, 96], f32, tag="tmp")
                res = resp.tile([128, sz, 48], odt, tag="res")
                for g in range(-(-sz // GRP)):
                    gsz = min(GRP, sz - g * GRP)
                    ps = psp.tile([128, gsz, 256], f32, tag="ps")
                    for bb in range(gsz):
                        k = g * GRP + bb
                        nc.tensor.matmul(
                            ps[:, bb, 0:32],
                            xtb_t[:, k * 128:(k + 1) * 128],
                            wb_t[:, rho, :])
                        nc.tensor.matmul(
                            ps[:, bb, 32:144],
                            xta_t[:, k * 128:(k + 1) * 128],
                            wa_t[:, rho, :])
                    gsl = slice(g * GRP, g * GRP + gsz)
                    nc.scalar.activation(
                        gates[:, gsl, 0:144], ps[:, :, 0:144], AF.Sigmoid)
                    if CGRP:
                        cchain(gates, auxt, tmp, res, gsl, tanh=True)
                if not CGRP:
                    cchain(gates, auxt, tmp, res, slice(0, sz), tanh=False)
                return (gates, auxt, tmp, res, blk0, sz)

            def cchain(gates, auxt, tmp, res, gs, tanh):
                # a = 2*sigmoid(2z) - 1 fixup (single-input, 2x on DVE)
                V.tensor_scalar(tmp[:, gs, S5], gates[:, gs, A], 2.0,
                                -1.0, ALU.mult, ALU.add)
                EC.tensor_mul(tmp[:, gs, S0], gates[:, gs, I], tmp[:, gs, S5])
                EC.tensor_mul(tmp[:, gs, S1], gates[:, gs, F],
                              auxt[:, gs, 0:16])
                EC.tensor_add(res[:, gs, RC], tmp[:, gs, S0], tmp[:, gs, S1])
                if tanh:
                    nc.scalar.activation(tmp[:, gs, S2], res[:, gs, RC],
                                         AF.Tanh)

            def elem_phase(state):
                gates, auxt, tmp, res, blk0, sz = state
                if not CGRP:
                    nc.scalar.activation(tmp[:, :, S2], res[:, :, RC],
                                         AF.Tanh)
                V.tensor_mul(res[:, :, RH], gates[:, :, O], tmp[:, :, S2])
                E6.tensor_mul(tmp[:, :, S3], gates[:, :, R0],
                              auxt[:, :, CH0])
                E7.tensor_mul(tmp[:, :, S4], gates[:, :, R1],
                              auxt[:, :, CH1])
                E8.tensor_mul(tmp[:, :, S0], gates[:, :, R2],
                              auxt[:, :, CH2])
                E9.tensor_add(tmp[:, :, S1], tmp[:, :, S3], tmp[:, :, S4])
                E10.tensor_add(tmp[:, :, S3], tmp[:, :, S1], tmp[:, :, S0])
                V.tensor_mul(tmp[:, :, S4], gates[:, :, N1], tmp[:, :, S3])
                V.tensor_mul(tmp[:, :, S0], gates[:, :, N2], res[:, :, RH])
                V.tensor_add(res[:, :, RN], tmp[:, :, S4], tmp[:, :, S0])
                # stores ride the ACT HWDGE ring so a store waiting on DVE
                # can't head-of-line-block the next supergroup's loads (SP ring)
                nc.scalar.dma_start(res_d[:, blk0:blk0 + sz, :], res[:])

            total = NSG * niter
            states = {}
            for s in range(total + LAG):
                if s - LAG >= 0 and (s - LAG) in states:
                    elem_phase(states.pop(s - LAG))
                if s < total:
                    states[s] = gate_phase(s % NSG)

    nc.compile()
    return nc


def _get_nc():
    if "nc" not in _NC_CACHE:
        _NC_CACHE["nc"] = build_nc()
    return _NC_CACHE["nc"]


def gather_out(results):
    n = np.empty((B, 16), np.float32)
    h = np.empty((B, 16), np.float32)
    c = np.empty((B, 16), np.float32)
    for m in range(NCORES):
        res = np.asarray(results[m]["res"]).astype(np.float32)
        flat = res.transpose(1, 0, 2).reshape(3 * TP, 48)
        for rho in range(3):
            first = m * R + ((rho - m * R) % 3)
            T = len(range(first, (m + 1) * R, 3))
            seg = flat[rho * TP: rho * TP + T]
            n[first:(m + 1) * R:3] = seg[:, 0:16]
            h[first:(m + 1) * R:3] = seg[:, 16:32]
            c[first:(m + 1) * R:3] = seg[:, 32:48]
    return n, h, c


def make_in_maps(cores):
    return [dict(xta=c["xta"], xtb=c["xtb"], wa=c["wa"], wb=c["wb"],
                 aux=c["aux"]) for c in cores]


def kernel(**inputs):
    from concourse.bass_utils import run_bass_kernel_spmd

    cores = host_prep(inputs)
    nc = _get_nc()
    out = run_bass_kernel_spmd(nc, make_in_maps(cores),
                               core_ids=list(range(NCORES)))
    return gather_out(out.results)

